# revision 20
# baseline (speedup 1.0000x reference)
"""Trainium2 Bass kernel for Gemma3 sliding-window attention.

Problem: B=1, T=4096, d_model=2048, 8 query heads / 4 KV heads, head_dim=256,
sliding window 1024, per-head RMSNorm + RoPE (interleaved rotate-half with
cat(freqs,freqs) tables), o_proj.

Sharding (8 cores): 4 KV-head groups x 2 sequence halves. Core (g, s) computes
query heads {2g, 2g+1} and KV head g for query tokens [s*2048, (s+1)*2048),
with a 1024-token KV halo (recomputed locally; s=0's halo is zero-padded and
masked out via the exp bias). Each core emits a partial o-projection
[2048, 2048]; the host sums the 4 group partials per half.

Dataflow: host pre-transposes x and all weights so every matmul operand loads
in its natural layout (contraction on partitions). Projections and attention
matmuls run in bf16 (fp32 PSUM accumulation); the RMSNorm/RoPE chain runs in
fp32/f32r (ssq via ones-matmul broadcast, rotate-half via a constant
permutation matmul). Attention is computed in S.T orientation per 512-query
block: S.T[j,i] = kT.T @ qT per 128-j tile, P.T = exp(S.T/16 + bias) on ACT
(bias -1e5 kills invalid j for the padded half), constant triangle masks on
the 8 window-edge tiles, softmax denominator via ones-matmul (no
max-subtraction: RMSNorm bounds |scores| <= 16), y.T = v.T @ P.T, then y.T is
scaled by the reciprocal denominator and consumed as lhsT by the o-projection.
"""

import sys

if "/opt/trn_rl_repo" not in sys.path:
    sys.path.insert(0, "/opt/trn_rl_repo")

import numpy as np

try:
    import ml_dtypes
    BF16 = ml_dtypes.bfloat16
except ImportError:
    BF16 = None

T, DM, NH, NKV, HD, WIN = 4096, 2048, 8, 4, 256, 1024
EPS, BASE = 1e-6, 10000.0
NG, NS = 4, 2
TL, NQ = 3072, 2048
NTT = 12          # 256-token projection tiles
QT0 = 4           # first query tile
NKO = 16          # 2048 / 128 contraction subtiles
NA = 4            # 512-query attention blocks per core
SCALE = 1.0 / 16.0
NEG = -1.0e5

_cache = {}


def _host_prep(x, pos, Wq, Wk, Wv, Wo, q_norm_w, k_norm_w):
    x = np.asarray(x, np.float32).reshape(T, DM)
    xT = np.ascontiguousarray(x.T)
    pos_f = np.asarray(pos).astype(np.float64)
    m = np.arange(128)
    invf = BASE ** (-m / 128.0)

    Wq = np.asarray(Wq, np.float32)
    Wk = np.asarray(Wk, np.float32)
    Wv = np.asarray(Wv, np.float32)
    Wo = np.asarray(Wo, np.float32)
    qnw = np.asarray(q_norm_w, np.float32)
    knw = np.asarray(k_norm_w, np.float32)

    ones = np.ones((128, 128), np.float32)
    r0T = np.zeros((128, 128), np.float32)
    a = np.arange(64)
    r0T[2 * a, 2 * a + 1] = 1.0
    r0T[2 * a + 1, 2 * a] = -1.0
    qw2 = np.ascontiguousarray(np.stack([qnw[:128], qnw[128:]], axis=1))
    kw2 = np.ascontiguousarray(np.stack([knw[:128], knw[128:]], axis=1))

    # masks for 512-wide attention blocks: m=0..3 far edge, m=8..11 diagonal
    jp = np.arange(128)[:, None]
    ip = np.arange(512)[None, :]
    tris = []
    for mm_ in range(4):
        tris.append(jp >= ip + 1 - 128 * mm_)         # far masks F_m
    for mm_ in range(4):
        tris.append(jp <= ip - 128 * mm_)             # diag masks D_{m+8}
    tri = np.concatenate(tris, axis=1).astype(BF16)   # [128, 8*512]

    in_maps = []
    for g in range(NG):
        for s in range(NS):
            lo = s * 2048 - 1024
            xT_c = np.zeros((DM, TL), np.float32)
            src_lo = max(lo, 0)
            xT_c[:, src_lo - lo:] = xT[:, src_lo:(s + 1) * 2048]
            pidx = np.clip(np.arange(lo, lo + TL), 0, T - 1)
            p = pos_f[pidx]
            p[np.arange(lo, lo + TL) < 0] = 0.0
            ang = p[None, :] * invf[:, None]
            cosk = np.ascontiguousarray(np.cos(ang), dtype=np.float32)
            sink = np.ascontiguousarray(np.sin(ang), dtype=np.float32)

            kbias = np.zeros((128, 24), np.float32)
            if s == 0:
                kbias[:, :8] = NEG

            in_maps.append({
                "xT": xT_c.astype(BF16),
                "cosk": cosk,
                "sink": sink,
                "wqT": np.ascontiguousarray(Wq[2 * g * HD:(2 * g + 2) * HD, :].T).astype(BF16),
                "wkT": np.ascontiguousarray(Wk[g * HD:(g + 1) * HD, :].T).astype(BF16),
                "wvT": np.ascontiguousarray(Wv[g * HD:(g + 1) * HD, :].T).astype(BF16),
                "woT": np.ascontiguousarray(Wo[:, 2 * g * HD:(2 * g + 2) * HD].T).astype(BF16),
                "ones_bf": ones.astype(BF16),
                "r0T": r0T.astype(BF16),
                "qw": qw2,
                "kw": kw2,
                "kbias": kbias,
                "tri": tri,
            })
    return in_maps


def _build_program():
    if "nc" in _cache:
        return _cache["nc"]

    import concourse.bass as bass
    import concourse.mybir as mybir
    import concourse.tile as tile
    from concourse import bacc
    from contextlib import ExitStack

    f32 = mybir.dt.float32
    f32r = mybir.dt.float32r
    bf16 = mybir.dt.bfloat16
    AF = mybir.ActivationFunctionType
    OP = mybir.AluOpType

    nc = bacc.Bacc("TRN2", target_bir_lowering=False, debug=False,
                   enable_asserts=False, num_devices=8)

    xT_d = nc.dram_tensor("xT", [DM, TL], bf16, kind="ExternalInput")
    cosk_d = nc.dram_tensor("cosk", [128, TL], f32, kind="ExternalInput")
    sink_d = nc.dram_tensor("sink", [128, TL], f32, kind="ExternalInput")
    wq_d = nc.dram_tensor("wqT", [DM, 512], bf16, kind="ExternalInput")
    wk_d = nc.dram_tensor("wkT", [DM, 256], bf16, kind="ExternalInput")
    wv_d = nc.dram_tensor("wvT", [DM, 256], bf16, kind="ExternalInput")
    wo_d = nc.dram_tensor("woT", [512, DM], bf16, kind="ExternalInput")
    onesbf_d = nc.dram_tensor("ones_bf", [128, 128], bf16, kind="ExternalInput")
    r0_d = nc.dram_tensor("r0T", [128, 128], bf16, kind="ExternalInput")
    qw_d = nc.dram_tensor("qw", [128, 2], f32, kind="ExternalInput")
    kw_d = nc.dram_tensor("kw", [128, 2], f32, kind="ExternalInput")
    kb_d = nc.dram_tensor("kbias", [128, 24], f32, kind="ExternalInput")
    tri_d = nc.dram_tensor("tri", [128, 8 * 512], bf16, kind="ExternalInput")
    o_d = nc.dram_tensor("o_part", [NQ, DM], f32, kind="ExternalOutput")

    def rv(ap):
        # f32 view of an f32r tile for DVE reads
        return ap.bitcast(f32)

    with tile.TileContext(nc) as tc, ExitStack() as ctx:
        cpool = ctx.enter_context(tc.tile_pool(name="consts", bufs=1))
        xpool = ctx.enter_context(tc.tile_pool(name="xt", bufs=3))
        tabpool = ctx.enter_context(tc.tile_pool(name="tab", bufs=2))
        kpool = ctx.enter_context(tc.tile_pool(name="kring", bufs=4))
        vpool = ctx.enter_context(tc.tile_pool(name="vring", bufs=4))
        scpool = ctx.enter_context(tc.tile_pool(name="scratch", bufs=3))
        spool = ctx.enter_context(tc.tile_pool(name="small", bufs=2))
        qpool = ctx.enter_context(tc.tile_pool(name="qt", bufs=2))
        ptpool = ctx.enter_context(tc.tile_pool(name="pt", bufs=4))
        ypool = ctx.enter_context(tc.tile_pool(name="yt", bufs=2))
        opool = ctx.enter_context(tc.tile_pool(name="osb", bufs=3))
        pp_proj = ctx.enter_context(tc.tile_pool(name="pproj", bufs=2, space="PSUM"))
        pp_small = ctx.enter_context(tc.tile_pool(name="psmall", bufs=2, space="PSUM"))
        pp_acc = ctx.enter_context(tc.tile_pool(name="pacc", bufs=4, space="PSUM"))

        # ---- resident constants / weights ----
        # (order matters: the first tile's k/v projections need wk/wv; wq is
        # needed at tt=4 and wo only at the first attention block)
        wk_sb = cpool.tile([128, NKO, 256], bf16, tag="wk")
        nc.sync.dma_start(wk_sb[:], wk_d.ap().rearrange("(ko p) c -> p ko c", p=128))
        wv_sb = cpool.tile([128, NKO, 256], bf16, tag="wv")
        nc.sync.dma_start(wv_sb[:], wv_d.ap().rearrange("(ko p) c -> p ko c", p=128))
        ones_sb = cpool.tile([128, 128], bf16, tag="ones")
        nc.sync.dma_start(ones_sb[:], onesbf_d.ap())
        onesbf_sb = ones_sb
        r0_sb = cpool.tile([128, 128], bf16, tag="r0")
        nc.sync.dma_start(r0_sb[:], r0_d.ap())
        qw_sb = cpool.tile([128, 2], f32, tag="qwt")
        nc.sync.dma_start(qw_sb[:], qw_d.ap())
        kw_sb = cpool.tile([128, 2], f32, tag="kwt")
        nc.sync.dma_start(kw_sb[:], kw_d.ap())
        kb_sb = cpool.tile([128, 24], f32, tag="kb")
        nc.sync.dma_start(kb_sb[:], kb_d.ap())
        eps_sb = cpool.tile([128, 1], f32, tag="eps")
        nc.vector.memset(eps_sb[:], EPS)
        zero_sb = cpool.tile([128, 1], f32, tag="zero")
        nc.vector.memset(zero_sb[:], 0.0)
        xT_v0 = xT_d.ap().rearrange("(ko p) t -> p ko t", p=128)
        pre_x = []
        for half in range(2):
            xt0 = xpool.tile([128, 8, 512], bf16, tag="xt")
            nc.sync.dma_start(xt0[:], xT_v0[:, half * 8:(half + 1) * 8, 0:512])
            pre_x.append(xt0)
        pre_cos = tabpool.tile([128, 512], f32, tag="cos")
        nc.sync.dma_start(pre_cos[:], cosk_d.ap()[:, 0:512])
        pre_sin = tabpool.tile([128, 512], f32, tag="sin")
        nc.sync.dma_start(pre_sin[:], sink_d.ap()[:, 0:512])
        wq_sb = cpool.tile([128, NKO, 512], bf16, tag="wq")
        nc.sync.dma_start(wq_sb[:], wq_d.ap().rearrange("(ko p) c -> p ko c", p=128))
        tri_sb = cpool.tile([128, 8 * 512], bf16, tag="tri")
        nc.sync.dma_start(tri_sb[:], tri_d.ap())
        wo_sb = cpool.tile([128, 4, DM], bf16, tag="wo")
        nc.sync.dma_start(wo_sb[:], wo_d.ap().rearrange("(hd p) c -> p hd c", p=128))

        xT_v = xT_d.ap().rearrange("(ko p) t -> p ko t", p=128)  # [128, 16, TL]

        NTB = 6                     # 512-token projection tiles
        kt_tiles = [None] * NTB
        vt_tiles = [None] * NTB

        def norm_rope(src_ps, w_sb, cos_t, sin_t, dst, dsti):
            """src_ps: two PSUM [128, 512] tiles (one head's 2 d-subtiles),
            transposed projection over 512 tokens. Writes RMSNorm+RoPE (bf16)
            into dst[:, dsti+u, :]."""
            z2 = scpool.tile([128, 2, 512], bf16, tag="z2")
            for u in range(2):
                nc.scalar.activation(z2[:, u, :], src_ps[u][:], AF.Square,
                                     bias=zero_sb[:])
            ssq = pp_small.tile([128, 512], f32, tag="psm")
            for u in range(2):
                nc.tensor.matmul(ssq[:], ones_sb[:], z2[:, u, :],
                                 start=(u == 0), stop=(u == 1))
            sq = spool.tile([128, 512], f32, tag="sq")
            nc.scalar.activation(sq[:], ssq[:], AF.Sqrt, bias=eps_sb[:], scale=1.0 / HD)
            rs = spool.tile([128, 512], f32, tag="rs")
            nc.vector.reciprocal_approx_fast(rs[:], sq[:])
            znw = scpool.tile([128, 2, 512], bf16, tag="znw")
            t1 = scpool.tile([128, 2, 512], f32, tag="t1")
            for u in range(2):
                nc.vector.scalar_tensor_tensor(
                    znw[:, u, :], src_ps[u][:], w_sb[:, u:u + 1], rs[:],
                    OP.mult, OP.mult)
                rot = pp_small.tile([128, 512], f32, tag="psm")
                nc.tensor.matmul(rot[:], r0_sb[:], znw[:, u, :], start=True, stop=True)
                nc.vector.tensor_tensor(t1[:, u, :], znw[:, u, :], cos_t, OP.mult)
                tmp = spool.tile([128, 512], f32, tag="tmp")
                nc.vector.tensor_tensor(tmp[:], rot[:], sin_t, OP.mult)
                nc.vector.tensor_tensor(dst[:, dsti + u, :], t1[:, u, :], tmp[:], OP.add)

        for tb in range(NTB):
            t0 = tb * 512
            if tb == 0:
                xth = pre_x
                cos_t = pre_cos
                sin_t = pre_sin
            else:
                xth = []
                for half in range(2):
                    xt = xpool.tile([128, 8, 512], bf16, tag="xt")
                    nc.sync.dma_start(xt[:], xT_v[:, half * 8:(half + 1) * 8, t0:t0 + 512])
                    xth.append(xt)
                cos_t = tabpool.tile([128, 512], f32, tag="cos")
                nc.sync.dma_start(cos_t[:], cosk_d.ap()[:, t0:t0 + 512])
                sin_t = tabpool.tile([128, 512], f32, tag="sin")
                nc.sync.dma_start(sin_t[:], sink_d.ap()[:, t0:t0 + 512])

            # ---- k projection (transposed, N=512) ----
            k0_ps = pp_proj.tile([128, 512], f32, tag="pj")
            k1_ps = pp_proj.tile([128, 512], f32, tag="pj")
            k_ps = [k0_ps, k1_ps]
            for dsub in range(2):
                for ko in range(NKO):
                    nc.tensor.matmul(k_ps[dsub][:],
                                     wk_sb[:, ko, dsub * 128:(dsub + 1) * 128],
                                     xth[ko // 8][:, ko % 8, :],
                                     start=(ko == 0), stop=(ko == NKO - 1))
            kt = kpool.tile([128, 2, 512], bf16, tag="kt")
            norm_rope(k_ps, kw_sb, cos_t[:], sin_t[:], kt, 0)
            kt_tiles[tb] = kt

            # ---- v projection (natural layout) ----
            vt = vpool.tile([128, 4, 256], bf16, tag="vt")
            for vh in range(2):
                v_ps = pp_proj.tile([128, 2, 256], f32, tag="pj")
                for ms in range(2):
                    msub = vh * 2 + ms
                    for ko in range(NKO):
                        nc.tensor.matmul(v_ps[:, ms, :],
                                         xth[ko // 8][:, ko % 8, msub * 128:(msub + 1) * 128],
                                         wv_sb[:, ko, :],
                                         start=(ko == 0), stop=(ko == NKO - 1))
                for ms in range(2):
                    nc.vector.tensor_copy(vt[:, vh * 2 + ms, :], v_ps[:, ms, :])
            vt_tiles[tb] = vt

            if tb < 2:
                continue

            # ---- q projections (2 heads, N=512) ----
            qt_sb = qpool.tile([128, 4, 512], bf16, tag="q")
            for h in range(2):
                q0_ps = pp_proj.tile([128, 512], f32, tag="pj")
                q1_ps = pp_proj.tile([128, 512], f32, tag="pj")
                q_ps = [q0_ps, q1_ps]
                for u in range(2):
                    dsub = 2 * h + u
                    for ko in range(NKO):
                        nc.tensor.matmul(q_ps[u][:],
                                         wq_sb[:, ko, dsub * 128:(dsub + 1) * 128],
                                         xth[ko // 8][:, ko % 8, :],
                                         start=(ko == 0), stop=(ko == NKO - 1))
                norm_rope(q_ps, qw_sb, cos_t[:], sin_t[:], qt_sb, 2 * h)

            # ---- attention for 512-query block a ----
            a = tb - 2
            yt_sb = ypool.tile([128, 4, 512], bf16, tag="y")
            for h in range(2):
                dn_ps = pp_acc.tile([128, 512], f32, tag="pac")
                y0_ps = pp_acc.tile([128, 512], f32, tag="pac")
                y1_ps = pp_acc.tile([128, 512], f32, tag="pac")
                y_ps = [y0_ps, y1_ps]
                for mi, mrel in enumerate([3, 0, 1, 2] + list(range(4, 12))):
                    jt = 4 * a + mrel
                    ct, jh = jt // 4, jt % 4
                    ktc = kt_tiles[ct]
                    vtc = vt_tiles[ct]
                    # active query range: edge tiles are mostly masked
                    if mrel <= 2:
                        ia, ib = 0, 128 * (mrel + 1)
                    elif mrel >= 9:
                        ia, ib = 128 * (mrel - 8), 512
                    else:
                        ia, ib = 0, 512
                    pt = ptpool.tile([128, 512], bf16, tag="p")
                    st = pp_small.tile([128, 512], f32, tag="psm")
                    for u in range(2):
                        nc.tensor.matmul(st[:, ia:ib],
                                         ktc[:, u, jh * 128:(jh + 1) * 128],
                                         qt_sb[:, 2 * h + u, ia:ib],
                                         start=(u == 0), stop=(u == 1))
                    nc.scalar.activation(pt[:, ia:ib], st[:, ia:ib], AF.Exp,
                                         bias=kb_sb[:, jt:jt + 1], scale=SCALE)
                    if mrel < 4:
                        nc.vector.tensor_tensor(
                            pt[:, ia:ib], pt[:, ia:ib],
                            tri_sb[:, mrel * 512 + ia:mrel * 512 + ib], OP.mult)
                    elif mrel >= 8:
                        nc.vector.tensor_tensor(
                            pt[:, ia:ib], pt[:, ia:ib],
                            tri_sb[:, (mrel - 4) * 512 + ia:(mrel - 4) * 512 + ib],
                            OP.mult)
                    first, last = (mi == 0), (mrel == 11)
                    nc.tensor.matmul(dn_ps[:, ia:ib], onesbf_sb[:], pt[:, ia:ib],
                                     start=first, stop=last, skip_group_check=True)
                    for dh in range(2):
                        nc.tensor.matmul(y_ps[dh][:, ia:ib],
                                         vtc[:, jh, dh * 128:(dh + 1) * 128],
                                         pt[:, ia:ib], start=first, stop=last,
                                         skip_group_check=True)
                rc = spool.tile([128, 512], f32, tag="rc")
                nc.vector.reciprocal_approx_fast(rc[:], dn_ps[:])
                for dh in range(2):
                    nc.vector.tensor_tensor(yt_sb[:, 2 * h + dh, :],
                                            y_ps[dh][:], rc[:], OP.mult)

            # ---- partial o-projection for the 512-query block ----
            for msub in range(4):
                for dmh in range(2):
                    o_sb = opool.tile([128, 1024], f32, tag="o")
                    for dq in range(2):
                        c0 = (dmh * 2 + dq) * 512
                        o_ps = pp_small.tile([128, 512], f32, tag="psm")
                        for hd in range(4):
                            nc.tensor.matmul(o_ps[:],
                                             yt_sb[:, hd, msub * 128:(msub + 1) * 128],
                                             wo_sb[:, hd, c0:c0 + 512],
                                             start=(hd == 0), stop=(hd == 3))
                        nc.scalar.copy(o_sb[:, dq * 512:(dq + 1) * 512], o_ps[:])
                    r0_ = a * 512 + msub * 128
                    nc.sync.dma_start(o_d.ap()[r0_:r0_ + 128, dmh * 1024:(dmh + 1) * 1024],
                                      o_sb[:])

    nc.compile()
    _cache["nc"] = nc
    return nc


def _run(inputs, trace=False):
    from concourse.bass_utils import run_bass_kernel_spmd

    nc = _build_program()
    in_maps = _host_prep(**inputs)
    res = run_bass_kernel_spmd(nc, in_maps, core_ids=list(range(8)), trace=trace)
    full = np.zeros((T, DM), np.float32)
    for g in range(NG):
        for s in range(NS):
            full[s * 2048:(s + 1) * 2048] += res.results[g * 2 + s]["o_part"]
    return full.reshape(1, T, DM), res


def kernel(**inputs):
    return _run(inputs, trace=False)[0]


# revision 21
# speedup vs baseline: 1.1071x; 1.1071x over previous
"""Trainium2 Bass kernel for Gemma3 sliding-window attention.

Problem: B=1, T=4096, d_model=2048, 8 query heads / 4 KV heads, head_dim=256,
sliding window 1024, per-head RMSNorm + RoPE (interleaved rotate-half with
cat(freqs,freqs) tables), o_proj.

Sharding (8 cores): 4 KV-head groups x 2 sequence halves. Core (g, s) computes
query heads {2g, 2g+1} and KV head g for query tokens [s*2048, (s+1)*2048),
with a 1024-token KV halo (recomputed locally; s=0's halo is zero-padded and
masked out via the exp bias). Each core emits a partial o-projection
[2048, 2048]; the host sums the 4 group partials per half.

Dataflow: host pre-transposes x and all weights so every matmul operand loads
in its natural layout (contraction on partitions). Projections and attention
matmuls run in bf16 (fp32 PSUM accumulation); the RMSNorm/RoPE chain runs in
fp32/f32r (ssq via ones-matmul broadcast, rotate-half via a constant
permutation matmul). Attention is computed in S.T orientation per 512-query
block: S.T[j,i] = kT.T @ qT per 128-j tile, P.T = exp(S.T/16 + bias) on ACT
(bias -1e5 kills invalid j for the padded half), constant triangle masks on
the 8 window-edge tiles, softmax denominator via ones-matmul (no
max-subtraction: RMSNorm bounds |scores| <= 16), y.T = v.T @ P.T, then y.T is
scaled by the reciprocal denominator and consumed as lhsT by the o-projection.
"""

import sys

if "/opt/trn_rl_repo" not in sys.path:
    sys.path.insert(0, "/opt/trn_rl_repo")

import numpy as np

try:
    import ml_dtypes
    BF16 = ml_dtypes.bfloat16
except ImportError:
    BF16 = None

T, DM, NH, NKV, HD, WIN = 4096, 2048, 8, 4, 256, 1024
EPS, BASE = 1e-6, 10000.0
NG, NS = 4, 2
TL, NQ = 3072, 2048
NTT = 12          # 256-token projection tiles
QT0 = 4           # first query tile
NKO = 16          # 2048 / 128 contraction subtiles
NA = 4            # 512-query attention blocks per core
SCALE = 1.0 / 16.0
NEG = -1.0e5

_cache = {}


def _host_prep(x, pos, Wq, Wk, Wv, Wo, q_norm_w, k_norm_w):
    x = np.asarray(x, np.float32).reshape(T, DM)
    xT = np.ascontiguousarray(x.T)
    pos_f = np.asarray(pos).astype(np.float64)
    m = np.arange(128)
    invf = BASE ** (-m / 128.0)

    Wq = np.asarray(Wq, np.float32)
    Wk = np.asarray(Wk, np.float32)
    Wv = np.asarray(Wv, np.float32)
    Wo = np.asarray(Wo, np.float32)
    qnw = np.asarray(q_norm_w, np.float32)
    knw = np.asarray(k_norm_w, np.float32)

    ones = np.ones((128, 128), np.float32)
    r0T = np.zeros((128, 128), np.float32)
    a = np.arange(64)
    r0T[2 * a, 2 * a + 1] = 1.0
    r0T[2 * a + 1, 2 * a] = -1.0
    qw2 = np.ascontiguousarray(np.stack([qnw[:128], qnw[128:]], axis=1))
    kw2 = np.ascontiguousarray(np.stack([knw[:128], knw[128:]], axis=1))

    # masks for 512-wide attention blocks: m=0..3 far edge, m=8..11 diagonal
    jp = np.arange(128)[:, None]
    ip = np.arange(512)[None, :]
    tris = []
    for mm_ in range(4):
        tris.append(jp >= ip + 1 - 128 * mm_)         # far masks F_m
    for mm_ in range(4):
        tris.append(jp <= ip - 128 * mm_)             # diag masks D_{m+8}
    tri = np.concatenate(tris, axis=1).astype(BF16)   # [128, 8*512]

    in_maps = []
    for g in range(NG):
        for s in range(NS):
            lo = s * 2048 - 1024
            xT_c = np.zeros((DM, TL), np.float32)
            src_lo = max(lo, 0)
            xT_c[:, src_lo - lo:] = xT[:, src_lo:(s + 1) * 2048]
            pidx = np.clip(np.arange(lo, lo + TL), 0, T - 1)
            p = pos_f[pidx]
            p[np.arange(lo, lo + TL) < 0] = 0.0
            ang = p[None, :] * invf[:, None]
            cosk = np.ascontiguousarray(np.cos(ang), dtype=np.float32)
            sink = np.ascontiguousarray(np.sin(ang), dtype=np.float32)

            kbias = np.zeros((128, 24), np.float32)
            if s == 0:
                kbias[:, :8] = NEG

            in_maps.append({
                "xT": xT_c.astype(BF16),
                "cosk": cosk,
                "sink": sink,
                "wqT": np.ascontiguousarray(Wq[2 * g * HD:(2 * g + 2) * HD, :].T).astype(BF16),
                "wkT": np.ascontiguousarray(Wk[g * HD:(g + 1) * HD, :].T).astype(BF16),
                "wvT": np.ascontiguousarray(Wv[g * HD:(g + 1) * HD, :].T).astype(BF16),
                "woT": np.ascontiguousarray(Wo[:, 2 * g * HD:(2 * g + 2) * HD].T).astype(BF16),
                "ones_bf": ones.astype(BF16),
                "r0T": r0T.astype(BF16),
                "qw": qw2,
                "kw": kw2,
                "kbias": kbias,
                "tri": tri,
            })
    return in_maps


def _build_program():
    if "nc" in _cache:
        return _cache["nc"]

    import concourse.bass as bass
    import concourse.mybir as mybir
    import concourse.tile as tile
    from concourse import bacc
    from contextlib import ExitStack

    f32 = mybir.dt.float32
    f32r = mybir.dt.float32r
    bf16 = mybir.dt.bfloat16
    AF = mybir.ActivationFunctionType
    OP = mybir.AluOpType

    nc = bacc.Bacc("TRN2", target_bir_lowering=False, debug=False,
                   enable_asserts=False, num_devices=8)

    xT_d = nc.dram_tensor("xT", [DM, TL], bf16, kind="ExternalInput")
    cosk_d = nc.dram_tensor("cosk", [128, TL], f32, kind="ExternalInput")
    sink_d = nc.dram_tensor("sink", [128, TL], f32, kind="ExternalInput")
    wq_d = nc.dram_tensor("wqT", [DM, 512], bf16, kind="ExternalInput")
    wk_d = nc.dram_tensor("wkT", [DM, 256], bf16, kind="ExternalInput")
    wv_d = nc.dram_tensor("wvT", [DM, 256], bf16, kind="ExternalInput")
    wo_d = nc.dram_tensor("woT", [512, DM], bf16, kind="ExternalInput")
    onesbf_d = nc.dram_tensor("ones_bf", [128, 128], bf16, kind="ExternalInput")
    r0_d = nc.dram_tensor("r0T", [128, 128], bf16, kind="ExternalInput")
    qw_d = nc.dram_tensor("qw", [128, 2], f32, kind="ExternalInput")
    kw_d = nc.dram_tensor("kw", [128, 2], f32, kind="ExternalInput")
    kb_d = nc.dram_tensor("kbias", [128, 24], f32, kind="ExternalInput")
    tri_d = nc.dram_tensor("tri", [128, 8 * 512], bf16, kind="ExternalInput")
    o_d = nc.dram_tensor("o_part", [NQ, DM], f32, kind="ExternalOutput")

    def rv(ap):
        # f32 view of an f32r tile for DVE reads
        return ap.bitcast(f32)

    with tile.TileContext(nc) as tc, ExitStack() as ctx:
        cpool = ctx.enter_context(tc.tile_pool(name="consts", bufs=1))
        xpool = ctx.enter_context(tc.tile_pool(name="xt", bufs=3))
        tabpool = ctx.enter_context(tc.tile_pool(name="tab", bufs=2))
        kpool = ctx.enter_context(tc.tile_pool(name="kring", bufs=4))
        vpool = ctx.enter_context(tc.tile_pool(name="vring", bufs=4))
        scpool = ctx.enter_context(tc.tile_pool(name="scratch", bufs=3))
        spool = ctx.enter_context(tc.tile_pool(name="small", bufs=2))
        qpool = ctx.enter_context(tc.tile_pool(name="qt", bufs=2))
        ptpool = ctx.enter_context(tc.tile_pool(name="pt", bufs=4))
        ypool = ctx.enter_context(tc.tile_pool(name="yt", bufs=2))
        opool = ctx.enter_context(tc.tile_pool(name="osb", bufs=3))
        pp_proj = ctx.enter_context(tc.tile_pool(name="pproj", bufs=3, space="PSUM"))
        pp_small = ctx.enter_context(tc.tile_pool(name="psmall", bufs=2, space="PSUM"))
        pp_acc = ctx.enter_context(tc.tile_pool(name="pacc", bufs=3, space="PSUM"))

        # ---- resident constants / weights ----
        # (order matters: the first tile's k/v projections need wk/wv; wq is
        # needed at tt=4 and wo only at the first attention block)
        wk_sb = cpool.tile([128, NKO, 256], bf16, tag="wk")
        nc.sync.dma_start(wk_sb[:], wk_d.ap().rearrange("(ko p) c -> p ko c", p=128))
        wv_sb = cpool.tile([128, NKO, 256], bf16, tag="wv")
        nc.sync.dma_start(wv_sb[:], wv_d.ap().rearrange("(ko p) c -> p ko c", p=128))
        ones_sb = cpool.tile([128, 128], bf16, tag="ones")
        nc.sync.dma_start(ones_sb[:], onesbf_d.ap())
        onesbf_sb = ones_sb
        r0_sb = cpool.tile([128, 128], bf16, tag="r0")
        nc.sync.dma_start(r0_sb[:], r0_d.ap())
        qw_sb = cpool.tile([128, 2], f32, tag="qwt")
        nc.sync.dma_start(qw_sb[:], qw_d.ap())
        kw_sb = cpool.tile([128, 2], f32, tag="kwt")
        nc.sync.dma_start(kw_sb[:], kw_d.ap())
        kb_sb = cpool.tile([128, 24], f32, tag="kb")
        nc.sync.dma_start(kb_sb[:], kb_d.ap())
        eps_sb = cpool.tile([128, 1], f32, tag="eps")
        nc.vector.memset(eps_sb[:], EPS)
        zero_sb = cpool.tile([128, 1], f32, tag="zero")
        nc.vector.memset(zero_sb[:], 0.0)
        xT_v0 = xT_d.ap().rearrange("(ko p) t -> p ko t", p=128)
        pre_x = []
        for half in range(2):
            xt0 = xpool.tile([128, 8, 512], bf16, tag="xt")
            nc.sync.dma_start(xt0[:], xT_v0[:, half * 8:(half + 1) * 8, 0:512])
            pre_x.append(xt0)
        pre_cos = tabpool.tile([128, 512], f32, tag="cos")
        nc.sync.dma_start(pre_cos[:], cosk_d.ap()[:, 0:512])
        pre_sin = tabpool.tile([128, 512], f32, tag="sin")
        nc.sync.dma_start(pre_sin[:], sink_d.ap()[:, 0:512])
        wq_sb = cpool.tile([128, NKO, 512], bf16, tag="wq")
        nc.sync.dma_start(wq_sb[:], wq_d.ap().rearrange("(ko p) c -> p ko c", p=128))
        tri_sb = cpool.tile([128, 8 * 512], bf16, tag="tri")
        nc.sync.dma_start(tri_sb[:], tri_d.ap())
        wo_sb = cpool.tile([128, 4, DM], bf16, tag="wo")
        nc.sync.dma_start(wo_sb[:], wo_d.ap().rearrange("(hd p) c -> p hd c", p=128))

        xT_v = xT_d.ap().rearrange("(ko p) t -> p ko t", p=128)  # [128, 16, TL]

        NTB = 6                     # 512-token projection tiles
        kt_tiles = [None] * NTB
        vt_tiles = [None] * NTB

        def norm_rope(src_ps, w_sb, cos_t, sin_t, dst, dsti):
            """src_ps: two PSUM [128, 512] tiles (one head's 2 d-subtiles),
            transposed projection over 512 tokens. Writes RMSNorm+RoPE (bf16)
            into dst[:, dsti+u, :]."""
            z2 = scpool.tile([128, 2, 512], bf16, tag="z2")
            for u in range(2):
                nc.scalar.activation(z2[:, u, :], src_ps[u][:], AF.Square,
                                     bias=zero_sb[:])
            ssq = pp_small.tile([128, 512], f32, tag="psm")
            for u in range(2):
                nc.tensor.matmul(ssq[:], ones_sb[:], z2[:, u, :],
                                 start=(u == 0), stop=(u == 1))
            sq = spool.tile([128, 512], f32, tag="sq")
            nc.scalar.activation(sq[:], ssq[:], AF.Sqrt, bias=eps_sb[:], scale=1.0 / HD)
            rs = spool.tile([128, 512], f32, tag="rs")
            nc.vector.reciprocal_approx_fast(rs[:], sq[:])
            znw = scpool.tile([128, 2, 512], bf16, tag="znw")
            t1 = scpool.tile([128, 2, 512], f32, tag="t1")
            for u in range(2):
                nc.vector.scalar_tensor_tensor(
                    znw[:, u, :], src_ps[u][:], w_sb[:, u:u + 1], rs[:],
                    OP.mult, OP.mult)
                rot = pp_small.tile([128, 512], f32, tag="psm")
                nc.tensor.matmul(rot[:], r0_sb[:], znw[:, u, :], start=True, stop=True)
                nc.vector.tensor_tensor(t1[:, u, :], znw[:, u, :], cos_t, OP.mult)
                tmp = spool.tile([128, 512], f32, tag="tmp")
                nc.vector.tensor_tensor(tmp[:], rot[:], sin_t, OP.mult)
                nc.vector.tensor_tensor(dst[:, dsti + u, :], t1[:, u, :], tmp[:], OP.add)

        for tb in range(NTB):
            t0 = tb * 512
            if tb == 0:
                xth = pre_x
                cos_t = pre_cos
                sin_t = pre_sin
            else:
                xth = []
                for half in range(2):
                    xt = xpool.tile([128, 8, 512], bf16, tag="xt")
                    nc.sync.dma_start(xt[:], xT_v[:, half * 8:(half + 1) * 8, t0:t0 + 512])
                    xth.append(xt)
                cos_t = tabpool.tile([128, 512], f32, tag="cos")
                nc.sync.dma_start(cos_t[:], cosk_d.ap()[:, t0:t0 + 512])
                sin_t = tabpool.tile([128, 512], f32, tag="sin")
                nc.sync.dma_start(sin_t[:], sink_d.ap()[:, t0:t0 + 512])

            # ---- k projection (transposed, N=512) ----
            k0_ps = pp_proj.tile([128, 512], f32, tag="pj")
            k1_ps = pp_proj.tile([128, 512], f32, tag="pj")
            k_ps = [k0_ps, k1_ps]
            for dsub in range(2):
                for ko in range(NKO):
                    nc.tensor.matmul(k_ps[dsub][:],
                                     wk_sb[:, ko, dsub * 128:(dsub + 1) * 128],
                                     xth[ko // 8][:, ko % 8, :],
                                     start=(ko == 0), stop=(ko == NKO - 1))
            kt = kpool.tile([128, 2, 512], bf16, tag="kt")
            norm_rope(k_ps, kw_sb, cos_t[:], sin_t[:], kt, 0)
            kt_tiles[tb] = kt

            # ---- v projection (natural layout) ----
            vt = vpool.tile([128, 4, 256], bf16, tag="vt")
            for vh in range(2):
                v_ps = pp_proj.tile([128, 2, 256], f32, tag="pj")
                for ms in range(2):
                    msub = vh * 2 + ms
                    for ko in range(NKO):
                        nc.tensor.matmul(v_ps[:, ms, :],
                                         xth[ko // 8][:, ko % 8, msub * 128:(msub + 1) * 128],
                                         wv_sb[:, ko, :],
                                         start=(ko == 0), stop=(ko == NKO - 1))
                for ms in range(2):
                    nc.vector.tensor_copy(vt[:, vh * 2 + ms, :], v_ps[:, ms, :])
            vt_tiles[tb] = vt

            if tb < 2:
                continue

            # ---- q projections (2 heads, N=512) ----
            qt_sb = qpool.tile([128, 4, 512], bf16, tag="q")
            for h in range(2):
                q0_ps = pp_proj.tile([128, 512], f32, tag="pj")
                q1_ps = pp_proj.tile([128, 512], f32, tag="pj")
                q_ps = [q0_ps, q1_ps]
                for u in range(2):
                    dsub = 2 * h + u
                    for ko in range(NKO):
                        nc.tensor.matmul(q_ps[u][:],
                                         wq_sb[:, ko, dsub * 128:(dsub + 1) * 128],
                                         xth[ko // 8][:, ko % 8, :],
                                         start=(ko == 0), stop=(ko == NKO - 1))
                norm_rope(q_ps, qw_sb, cos_t[:], sin_t[:], qt_sb, 2 * h)

            # ---- attention for 512-query block a ----
            a = tb - 2
            yt_sb = ypool.tile([128, 4, 512], bf16, tag="y")
            for h in range(2):
                dn_ps = pp_acc.tile([128, 512], f32, tag="pac")
                y0_ps = pp_acc.tile([128, 512], f32, tag="pac")
                y1_ps = pp_acc.tile([128, 512], f32, tag="pac")
                y_ps = [y0_ps, y1_ps]
                for mi, mrel in enumerate([3, 0, 1, 2] + list(range(4, 12))):
                    jt = 4 * a + mrel
                    ct, jh = jt // 4, jt % 4
                    ktc = kt_tiles[ct]
                    vtc = vt_tiles[ct]
                    # active query range: edge tiles are mostly masked
                    if mrel <= 2:
                        ia, ib = 0, 128 * (mrel + 1)
                    elif mrel >= 9:
                        ia, ib = 128 * (mrel - 8), 512
                    else:
                        ia, ib = 0, 512
                    pt = ptpool.tile([128, 512], bf16, tag="p")
                    st = pp_small.tile([128, 512], f32, tag="psm")
                    for u in range(2):
                        nc.tensor.matmul(st[:, ia:ib],
                                         ktc[:, u, jh * 128:(jh + 1) * 128],
                                         qt_sb[:, 2 * h + u, ia:ib],
                                         start=(u == 0), stop=(u == 1))
                    nc.scalar.activation(pt[:, ia:ib], st[:, ia:ib], AF.Exp,
                                         bias=kb_sb[:, jt:jt + 1], scale=SCALE)
                    if mrel < 4:
                        nc.vector.tensor_tensor(
                            pt[:, ia:ib], pt[:, ia:ib],
                            tri_sb[:, mrel * 512 + ia:mrel * 512 + ib], OP.mult)
                    elif mrel >= 8:
                        nc.vector.tensor_tensor(
                            pt[:, ia:ib], pt[:, ia:ib],
                            tri_sb[:, (mrel - 4) * 512 + ia:(mrel - 4) * 512 + ib],
                            OP.mult)
                    first, last = (mi == 0), (mrel == 11)
                    nc.tensor.matmul(dn_ps[:, ia:ib], onesbf_sb[:], pt[:, ia:ib],
                                     start=first, stop=last, skip_group_check=True)
                    for dh in range(2):
                        nc.tensor.matmul(y_ps[dh][:, ia:ib],
                                         vtc[:, jh, dh * 128:(dh + 1) * 128],
                                         pt[:, ia:ib], start=first, stop=last,
                                         skip_group_check=True)
                rc = spool.tile([128, 512], f32, tag="rc")
                nc.vector.reciprocal_approx_fast(rc[:], dn_ps[:])
                for dh in range(2):
                    nc.vector.tensor_tensor(yt_sb[:, 2 * h + dh, :],
                                            y_ps[dh][:], rc[:], OP.mult)

            # ---- partial o-projection for the 512-query block ----
            for msub in range(4):
                for dmh in range(2):
                    o_sb = opool.tile([128, 1024], f32, tag="o")
                    for dq in range(2):
                        c0 = (dmh * 2 + dq) * 512
                        o_ps = pp_small.tile([128, 512], f32, tag="psm")
                        for hd in range(4):
                            nc.tensor.matmul(o_ps[:],
                                             yt_sb[:, hd, msub * 128:(msub + 1) * 128],
                                             wo_sb[:, hd, c0:c0 + 512],
                                             start=(hd == 0), stop=(hd == 3))
                        nc.scalar.copy(o_sb[:, dq * 512:(dq + 1) * 512], o_ps[:])
                    r0_ = a * 512 + msub * 128
                    nc.sync.dma_start(o_d.ap()[r0_:r0_ + 128, dmh * 1024:(dmh + 1) * 1024],
                                      o_sb[:])

    nc.compile()
    _cache["nc"] = nc
    return nc


def _run(inputs, trace=False):
    from concourse.bass_utils import run_bass_kernel_spmd

    nc = _build_program()
    in_maps = _host_prep(**inputs)
    res = run_bass_kernel_spmd(nc, in_maps, core_ids=list(range(8)), trace=trace)
    full = np.zeros((T, DM), np.float32)
    for g in range(NG):
        for s in range(NS):
            full[s * 2048:(s + 1) * 2048] += res.results[g * 2 + s]["o_part"]
    return full.reshape(1, T, DM), res


def kernel(**inputs):
    return _run(inputs, trace=False)[0]
